# revision 9
# baseline (speedup 1.0000x reference)
"""AntiPatternLoss Trainium2 kernel (8 NeuronCores, data-parallel over batch).

Reference computation (per batch row of logits [T=2048, V=128]):
  pred      = argmax_v(logits)                                    # [T]
  prob_pred = softmax(logits)[t, pred[t]] = 1 / sum_v exp(l - max)
  pen[j]    = mean_{k<3} prob_pred[j+k]                           # [L], L = T-2
  eq[i,j]   = (trigram at i == trigram at j) and (j - i >= 3)
  loss      = REP_PEN * sum_j(count_j * pen_j) / (B*T)  (0 if no pairs,
              which the sum already yields, so no predicate needed)

Kernel strategy per core (2 rows):
  - encode each trigram as one integer code = p0*128^2 + p1*128 + p2
    (< 2^21, exact in fp32) so pairwise matching is ONE equality compare
  - counts via fused DVE tensor_scalar(is_equal, accum_out=sum) per j-tile
    of 128 positions (j on partitions), upper triangle only
  - the partial diagonal 128x128 blocks are masked via min() with a
    precomputed staircase sentinel mask
  - per-core partial loss scalars are summed on the host (gather step)
"""

import numpy as np

import concourse.bass as bass
import concourse.mybir as mybir
from concourse import bacc, masks, tile
from concourse.bass_utils import run_bass_kernel_spmd

F32 = mybir.dt.float32
AL = mybir.AluOpType

N_CORES = 8
B, T, V = 16, 2048, 128
R = B // N_CORES          # rows per core = 2
NGRAM = 3
REP_PEN = 1.2
L = T - NGRAM + 1         # 2046 trigram start positions
NT = T // 128             # 16 j-tiles per row
PAD = 2                   # sentinel cols in front of codes (for diag s=0)
SENT_PAD = -1.0           # code_bcast padding sentinel
SENT_MASK = -2.0          # staircase-mask fill sentinel
SENT_J = -3.0             # sentinel code for positions without a trigram
SCALE = REP_PEN / (NGRAM * B * T)   # pen's /3 folded in
CB_W = PAD + T + 2        # code_bcast width (2050)


def build_nc():
    nc = bacc.Bacc("TRN2", target_bir_lowering=False, debug=False,
                   num_devices=N_CORES)
    x_ext = nc.dram_tensor("logits", [R * T, V], F32, kind="ExternalInput")
    y_ext = nc.dram_tensor("out", [1, 1], F32, kind="ExternalOutput")

    with tile.TileContext(nc) as tc:
        with (
            tc.tile_pool(name="setup", bufs=1) as setup,
            tc.tile_pool(name="big", bufs=1) as big,
            tc.tile_pool(name="small", bufs=1) as small,
            tc.tile_pool(name="scr", bufs=1) as scrp,
            tc.tile_pool(name="ps", bufs=1, space="PSUM") as ps,
            tc.tile_pool(name="psf", bufs=1, space="PSUM") as psf,
            tc.tile_pool(name="dram", bufs=1, space="DRAM") as dram,
        ):
            # ---------------- one-time setup (gpsimd; off the DVE path) ---
            ident = setup.tile([128, 128], F32)
            masks.make_identity(nc, ident[:])

            iota_v = setup.tile([128, 128], F32)  # iota_v[p, v] = v
            nc.gpsimd.iota(iota_v[:], pattern=[[1, 128]], base=0,
                           channel_multiplier=0,
                           allow_small_or_imprecise_dtypes=True)

            ones_col = setup.tile([128, 1], F32)
            nc.gpsimd.memset(ones_col[:], 1.0)

            # shift matrices: S1[k,m] = (k == m+1), S2[k,m] = (k == m+2)
            s1m = setup.tile([128, 128], F32)
            nc.gpsimd.memset(s1m[:], 1.0)
            nc.gpsimd.affine_select(out=s1m[:], in_=s1m[:],
                                    pattern=[[-1, 128]], compare_op=AL.is_equal,
                                    fill=0.0, base=-1, channel_multiplier=1)
            s2m = setup.tile([128, 128], F32)
            nc.gpsimd.memset(s2m[:], 1.0)
            nc.gpsimd.affine_select(out=s2m[:], in_=s2m[:],
                                    pattern=[[-1, 128]], compare_op=AL.is_equal,
                                    fill=0.0, base=-2, channel_multiplier=1)

            # staircase min-mask: keep (c < p) -> +BIG, else SENT_MASK
            maskmin = setup.tile([128, NT * 128], F32)
            nc.gpsimd.memset(maskmin[:], 3.0e38)
            nc.gpsimd.affine_select(out=maskmin[:].rearrange("p (s c) -> p s c", c=128),
                                    in_=maskmin[:].rearrange("p (s c) -> p s c", c=128),
                                    pattern=[[0, NT], [-1, 128]],
                                    compare_op=AL.is_gt, fill=SENT_MASK,
                                    base=0, channel_multiplier=1)

            # ---------------- per-row persistent tiles ----------------
            logits_sb = [big.tile([128, NT * 128], F32, tag=f"lg{r}", name=f"logits_sb{r}") for r in range(R)]
            code_bcast = [big.tile([128, CB_W], F32, tag=f"cb{r}", name=f"code_bcast{r}") for r in range(R)]
            diag_sc = [big.tile([128, NT * 128], F32, tag=f"dg{r}", name=f"diag_sc{r}") for r in range(R)]

            # merged [128, 16*R] per-position tiles (col = 16*r + n)
            max8 = small.tile([128, 8 * NT * R], F32)
            idx8 = small.tile([128, 8 * NT * R], mybir.dt.uint32)
            negmax2 = small.tile([128, NT * R], F32)
            pred2 = small.tile([128, NT * R], F32)
            sumexp2 = small.tile([128, NT * R], F32)
            pp2 = small.tile([128, NT * R], F32)
            sh1 = small.tile([128, NT * R], F32)
            sh2 = small.tile([128, NT * R], F32)
            psh1 = small.tile([128, NT * R], F32)
            psh2 = small.tile([128, NT * R], F32)
            tmp_a = small.tile([128, NT * R], F32)
            tmp_b = small.tile([128, NT * R], F32)
            code2 = small.tile([128, NT * R], F32)
            pen2 = small.tile([128, NT * R], F32)
            counts_a = small.tile([128, NT * R], F32)
            counts_b = small.tile([128, NT * R], F32)
            s1col = small.tile([128, 1], F32)

            mscr = scrp.tile([128, 1920], F32)        # main-loop out scratch
            dscr = scrp.tile([128, 128], F32)         # diag out scratch
            junk16 = scrp.tile([128, NT * R], F32)
            junk2 = scrp.tile([1, 1], F32)
            final_sb = scrp.tile([1, 1], F32)
            exp_scr = [scrp.tile([128, 128], F32, tag=f"ex{i}", name=f"exp_scr{i}") for i in range(2)]

            code_dram = dram.tile([1, R * T], F32)

            x = x_ext.ap()

            # ---------------- preprocessing (per row) ----------------
            for r in range(R):
                # logits[r] as [128 part = t%128, n = t//128, v]
                src = x[r * T:(r + 1) * T, :].rearrange("(n p) v -> p n v", p=128)
                nc.sync.dma_start(logits_sb[r][:].rearrange("p (n v) -> p n v", v=128), src)

                lg3 = logits_sb[r][:].rearrange("p (n v) -> p n v", v=128)
                c0 = NT * r
                # top-8 + first-match index per position (exact argmax ties)
                for n in range(NT):
                    c = c0 + n
                    nc.vector.max(max8[:, 8 * c:8 * c + 8], lg3[:, n, :])
                    nc.vector.max_index(idx8[:, 8 * c:8 * c + 8],
                                        max8[:, 8 * c:8 * c + 8], lg3[:, n, :])
                # negmax for the exp bias (strided view over top-1 column)
                m8v = max8[:].rearrange("p (c e) -> p c e", e=8)
                nc.vector.tensor_scalar(out=negmax2[:, c0:c0 + NT],
                                        in0=m8v[:, c0:c0 + NT, 0],
                                        scalar1=-1.0, scalar2=None, op0=AL.mult)
                for n in range(NT):
                    c = c0 + n
                    # exp(l - max) with fused sum (ScalarE)
                    nc.scalar.activation(exp_scr[n % 2][:], lg3[:, n, :],
                                         mybir.ActivationFunctionType.Exp,
                                         bias=negmax2[:, c:c + 1], scale=1.0,
                                         accum_out=sumexp2[:, c:c + 1])

            # pred as f32 (cast from the top-1 index column)
            i8v = idx8[:].rearrange("p (c e) -> p c e", e=8)
            nc.vector.tensor_copy(pred2[:], i8v[:, :, 0])

            # prob_pred = 1 / sumexp (both rows at once)
            nc.vector.reciprocal(pp2[:], sumexp2[:])

            # ---------------- trigram codes + pen (merged rows) ----------
            ps_sh1 = ps.tile([128, NT * R], F32)
            ps_sh2 = ps.tile([128, NT * R], F32)
            ps_ph1 = ps.tile([128, NT * R], F32)
            ps_ph2 = ps.tile([128, NT * R], F32)
            nc.tensor.matmul(ps_sh1[:], s1m[:], pred2[:], start=True, stop=True)
            nc.tensor.matmul(ps_sh2[:], s2m[:], pred2[:], start=True, stop=True)
            nc.tensor.matmul(ps_ph1[:], s1m[:], pp2[:], start=True, stop=True)
            nc.tensor.matmul(ps_ph2[:], s2m[:], pp2[:], start=True, stop=True)
            nc.vector.tensor_copy(sh1[:], ps_sh1[:])
            nc.vector.tensor_copy(sh2[:], ps_sh2[:])
            nc.vector.tensor_copy(psh1[:], ps_ph1[:])
            nc.vector.tensor_copy(psh2[:], ps_ph2[:])
            # partition-wrap fixups: value at (p=127, n) lives at (p=0, n+1)
            nc.sync.dma_start(sh1[127:128, 0:NT * R - 1], pred2[0:1, 1:NT * R])
            nc.sync.dma_start(sh2[126:128, 0:NT * R - 1], pred2[0:2, 1:NT * R])
            nc.sync.dma_start(psh1[127:128, 0:NT * R - 1], pp2[0:1, 1:NT * R])
            nc.sync.dma_start(psh2[126:128, 0:NT * R - 1], pp2[0:2, 1:NT * R])

            # code = pred*16384 + sh1*128 + sh2
            nc.vector.tensor_scalar(out=tmp_a[:], in0=pred2[:], scalar1=16384.0,
                                    scalar2=None, op0=AL.mult)
            nc.vector.scalar_tensor_tensor(out=tmp_b[:], in0=sh1[:], scalar=128.0,
                                           in1=tmp_a[:], op0=AL.mult, op1=AL.add)
            nc.vector.tensor_tensor(out=code2[:], in0=tmp_b[:], in1=sh2[:], op=AL.add)
            # positions T-2, T-1 have no trigram -> sentinel (never matches).
            # engine ops can't start at partition 126, so write via DMA.
            sent_tile = small.tile([2, 1], F32)
            nc.gpsimd.memset(sent_tile[:], SENT_J)
            for r in range(R):
                nc.sync.dma_start(code2[126:128, NT * (r + 1) - 1:NT * (r + 1)],
                                  sent_tile[:])

            # pen = pp + pp_shift1 + pp_shift2   (the /3 is folded into SCALE)
            nc.vector.tensor_tensor(out=pen2[:], in0=pp2[:], in1=psh1[:], op=AL.add)
            nc.vector.tensor_tensor(out=pen2[:], in0=pen2[:], in1=psh2[:], op=AL.add)

            # ---------------- broadcast codes to all partitions ----------
            ps_ct = ps.tile([NT * R, 128], F32)
            nc.tensor.transpose(ps_ct[:], code2[:], ident[:])
            codeT_sb = small.tile([NT * R, 128], F32)
            nc.vector.tensor_copy(codeT_sb[:], ps_ct[:])
            # flatten [32, 128] -> DRAM [1, 4096] (row-major = t order per row)
            nc.sync.dma_start(code_dram[:].rearrange("o (q p) -> o q p", p=128)[0],
                              codeT_sb[:])
            for r in range(R):
                nc.vector.memset(code_bcast[r][:, 0:PAD], SENT_PAD)
                nc.sync.dma_start(
                    code_bcast[r][:, PAD:PAD + T],
                    code_dram[:, r * T:(r + 1) * T].partition_broadcast(128))

            # ---------------- pairwise match counting ----------------
            for r in range(R):
                c0 = NT * r
                # masked staircase for the 16 partial-diagonal blocks:
                # stair[p, s, c] = code[128*s - 2 + c]; keep only c < p
                stair = code_bcast[r][:, PAD - 2:PAD - 2 + NT * 128] \
                    .rearrange("p (s c) -> p s c", c=128)
                nc.vector.tensor_tensor(out=diag_sc[r][:].rearrange("p (s c) -> p s c", c=128),
                                        in0=stair, in1=maskmin[:].rearrange("p (s c) -> p s c", c=128),
                                        op=AL.min)
                for s in range(NT):
                    W = 128 * s - 2
                    if W > 0:
                        # full i-window [0, 128s-2) valid for every p
                        nc.vector.tensor_scalar(
                            out=mscr[:, 0:W], in0=code_bcast[r][:, PAD:PAD + W],
                            scalar1=code2[:, c0 + s:c0 + s + 1], scalar2=None,
                            op0=AL.is_equal, op1=AL.add,
                            accum_out=counts_a[:, c0 + s:c0 + s + 1])
                    # diagonal block, chained accumulate via scalar2
                    nc.vector.tensor_scalar(
                        out=dscr[:], in0=diag_sc[r][:, 128 * s:128 * s + 128],
                        scalar1=code2[:, c0 + s:c0 + s + 1],
                        scalar2=(counts_a[:, c0 + s:c0 + s + 1] if W > 0 else None),
                        op0=AL.is_equal, op1=AL.add,
                        accum_out=counts_b[:, c0 + s:c0 + s + 1])

            # ---------------- epilogue ----------------
            # s1col[p] = sum_s counts * pen   (both rows at once)
            nc.vector.scalar_tensor_tensor(out=junk16[:], in0=counts_b[:],
                                           scalar=0.0, in1=pen2[:],
                                           op0=AL.add, op1=AL.mult,
                                           accum_out=s1col[:])
            ps_fin = psf.tile([1, 1], F32)
            nc.tensor.matmul(ps_fin[:], ones_col[:], s1col[:], start=True, stop=True)
            nc.vector.tensor_scalar(out=junk2[:], in0=ps_fin[:], scalar1=SCALE,
                                    scalar2=None, op0=AL.mult, op1=AL.add,
                                    accum_out=final_sb[:])
            nc.sync.dma_start(y_ext.ap()[:, :], final_sb[:])

    nc.compile()
    return nc


_NC_CACHE = None


def _get_nc():
    global _NC_CACHE
    if _NC_CACHE is None:
        _NC_CACHE = build_nc()
    return _NC_CACHE


def kernel(**inputs) -> np.ndarray:
    logits = np.ascontiguousarray(np.asarray(inputs["logits"], dtype=np.float32))
    assert logits.shape == (B, T, V), logits.shape
    nc = _get_nc()
    in_maps = [
        {"logits": logits[i * R:(i + 1) * R].reshape(R * T, V)}
        for i in range(N_CORES)
    ]
    res = run_bass_kernel_spmd(nc, in_maps, core_ids=list(range(N_CORES)))
    total = np.float32(0.0)
    for i in range(N_CORES):
        total = total + res.results[i]["out"][0, 0]
    return np.asarray(total, dtype=np.float32)


# revision 15
# speedup vs baseline: 1.2857x; 1.2857x over previous
"""AntiPatternLoss Trainium2 kernel (8 NeuronCores, data-parallel over batch).

Reference computation (per batch row of logits [T=2048, V=128]):
  pred      = argmax_v(logits)                                    # [T]
  prob_pred = softmax(logits)[t, pred[t]] = 1 / sum_v exp(l - max)
  pen[j]    = mean_{k<3} prob_pred[j+k]                           # [L], L = T-2
  eq[i,j]   = (trigram at i == trigram at j) and (j - i >= 3)
  loss      = REP_PEN * sum_j(count_j * pen_j) / (B*T)   (no-pair case
              yields 0 through the sum already, so no predicate needed)

Kernel strategy per core (2 rows):
  - logits loaded contiguously as [128, 16, 128] with partition = t//16
  - exact tie-faithful argmax: rowmax -> eq=(l==max) -> eq*(127-v) ->
    reduce-max -> 127-red (picks the FIRST max index like jnp.argmax)
  - trigram code = p0*16384 + p1*128 + p2 (< 2^21, exact in fp32);
    pairwise match is ONE fp32 equality compare
  - main O(L^2) loop: i on partitions, per i-tile a fused DVE
    tensor_scalar(is_equal) at 2x (bf16 out, no accum); the i-reduction
    runs on TensorE as ones-matmul accumulating counts[1, j] in PSUM
  - partial diagonal blocks: per-tile eq + one staircase mask multiply
  - per-core partial loss scalars are summed on the host (gather step)
"""

import numpy as np

import concourse.bass as bass
import concourse.mybir as mybir
from concourse import bacc, tile
from concourse.bass_utils import run_bass_kernel_spmd

F32 = mybir.dt.float32
BF16 = mybir.dt.bfloat16
AL = mybir.AluOpType
AF = mybir.ActivationFunctionType

N_CORES = 8
B, T, V = 16, 2048, 128
R = B // N_CORES          # rows per core = 2
NGRAM = 3
REP_PEN = 1.2
L = T - NGRAM + 1         # 2046 trigram start positions
NT = T // 128             # 16 i-tiles per row
PAD = 2                   # sentinel cols in front of codes in code_bcast
SENT_BC = -1.0            # j-side (code_bcast / code flat) sentinel
SENT_I = -3.0             # i-side (code_ipart) sentinel
SCALE = REP_PEN / (NGRAM * B * T)   # pen's /3 folded in
CB_W = PAD + T + 8        # code_bcast width


def _bank_chunks(a, b):
    """Split [a, b) at 512-column PSUM bank boundaries."""
    out = []
    while a < b:
        nxt = min(b, (a // 512 + 1) * 512)
        out.append((a, nxt))
        a = nxt
    return out


def build_nc():
    nc = bacc.Bacc("TRN2", target_bir_lowering=False, debug=False,
                   num_devices=N_CORES)
    x_ext = nc.dram_tensor("logits", [R * T, V], F32, kind="ExternalInput")
    y_ext = nc.dram_tensor("out", [1, 1], F32, kind="ExternalOutput")

    with tile.TileContext(nc) as tc:
        with (
            tc.tile_pool(name="setup", bufs=1) as setup,
            tc.tile_pool(name="big", bufs=1) as big,
            tc.tile_pool(name="small", bufs=1) as small,
            tc.tile_pool(name="eqp", bufs=4) as eqp,
            tc.tile_pool(name="scr", bufs=1) as scrp,
            tc.tile_pool(name="ps", bufs=1, space="PSUM") as ps,
            tc.tile_pool(name="dram", bufs=1, space="DRAM") as dram,
        ):
            # ---------------- one-time setup (gpsimd; off the DVE path) ---
            wrev = setup.tile([128, 128], BF16)   # wrev[p, v] = 127 - v
            nc.gpsimd.iota(wrev[:], pattern=[[-1, 128]], base=127,
                           channel_multiplier=0,
                           allow_small_or_imprecise_dtypes=True)

            ones_bf = setup.tile([128, 1], BF16)
            nc.gpsimd.memset(ones_bf[:], 1.0)
            ones_f32 = setup.tile([128, 1], F32)
            nc.gpsimd.memset(ones_f32[:], 1.0)

            # diag mask[p, t, c] = 1 if (c >= p and c <= 126) else 0
            diagmask = setup.tile([128, NT * 128], BF16)
            dm3 = diagmask[:].rearrange("p (t c) -> p t c", c=128)
            nc.gpsimd.memset(diagmask[:], 1.0)
            nc.gpsimd.affine_select(out=dm3, in_=dm3,
                                    pattern=[[0, NT], [1, 128]],
                                    compare_op=AL.is_ge, fill=0.0,
                                    base=0, channel_multiplier=-1)
            nc.gpsimd.affine_select(out=dm3, in_=dm3,
                                    pattern=[[0, NT], [-1, 128]],
                                    compare_op=AL.is_ge, fill=0.0,
                                    base=126, channel_multiplier=0)

            sentI = setup.tile([2, 1], F32)
            nc.gpsimd.memset(sentI[:], SENT_I)
            zeros_bf = setup.tile([1, 8], BF16)
            nc.gpsimd.memset(zeros_bf[:], 0.0)

            # ---------------- tiles ----------------
            logits_sb = [big.tile([128, NT * 128], F32, tag=f"lg{r}",
                                  name=f"logits_sb{r}") for r in range(R)]
            code_bcast = [big.tile([128, CB_W], F32, tag=f"cb{r}",
                                   name=f"code_bcast{r}") for r in range(R)]
            eqbig = [big.tile([128, NT * 128], BF16, tag=f"eqd{r}",
                              name=f"eqd{r}") for r in range(R)]  # diag eq
            m3b = [big.tile([128, NT * 128], BF16, tag=f"m3{r}",
                            name=f"m3b{r}") for r in range(R)]
            eqb = [big.tile([128, NT * 128], BF16, tag=f"eq3{r}",
                            name=f"eqb{r}") for r in range(R)]

            C2 = NT * R   # 32 merged columns (col = 16r + n)
            rowmax2 = small.tile([128, C2], F32)
            negmax2 = small.tile([128, C2], F32)
            red2 = small.tile([128, C2], BF16)
            pred2 = small.tile([128, C2], F32)
            sumexp2 = small.tile([128, C2], F32)
            pp2 = small.tile([128, C2], F32)
            sh1 = small.tile([128, C2], F32)
            sh2 = small.tile([128, C2], F32)
            ph1 = small.tile([128, C2], F32)
            ph2 = small.tile([128, C2], F32)
            tmp_a = small.tile([128, C2], F32)
            tmp_b = small.tile([128, C2], F32)
            code2 = small.tile([128, C2], F32)
            pen2 = small.tile([128, C2], F32)
            pred_nxt = small.tile([128, 2 * R], F32)
            pp_nxt = small.tile([128, 2 * R], F32)
            pred_bf = small.tile([128, C2], BF16)
            p0t = small.tile([128, C2], BF16)
            p1t = small.tile([128, C2], BF16)
            p2t = small.tile([128, C2], BF16)
            code_ipart = small.tile([128, C2], F32)
            counts_div = small.tile([128, NT], F32)
            counts_sb = small.tile([1, T], F32)
            s1c = small.tile([128, R], F32)
            junk16 = scrp.tile([128, NT], F32)
            junk2 = scrp.tile([1, R], F32)
            final_sb = scrp.tile([1, 1], F32)
            exp_scr = [scrp.tile([128, 128], F32, tag=f"ex{i}",
                                 name=f"exp_scr{i}") for i in range(2)]

            # row r's counts live at partition 32*r (matmul out base must be 0/32/64)
            counts_ps = ps.tile([32 * (R - 1) + 1, T], F32)
            ps_fin = ps.tile([1, R], F32)

            code_flat = dram.tile([1, R * T], F32)
            pred_flat = dram.tile([1, R * T + 256], BF16)

            x = x_ext.ap()

            # ---------------- load + preprocessing (per row) -------------
            lg3 = []
            for r in range(R):
                # contiguous: partition p holds t in [16p, 16p+16)
                src = x[r * T:(r + 1) * T, :].rearrange("(a b) v -> a (b v)", a=128)
                nc.sync.dma_start(logits_sb[r][:], src)
                lg3.append(logits_sb[r][:].rearrange("p (b v) -> p b v", v=128))

            for r in range(R):
                c0 = NT * r
                nc.vector.tensor_reduce(out=rowmax2[:, c0:c0 + NT], in_=lg3[r],
                                        axis=mybir.AxisListType.X, op=AL.max)
                rm_b = rowmax2[:, c0:c0 + NT] \
                    .rearrange("p (b o) -> p b o", o=1).to_broadcast((128, NT, 128))
                eq3 = eqb[r][:].rearrange("p (b v) -> p b v", v=128)
                nc.vector.tensor_tensor(out=eq3, in0=lg3[r], in1=rm_b,
                                        op=AL.is_equal)
                wrev_b = wrev[:].rearrange("p (o v) -> p o v", o=1) \
                    .to_broadcast((128, NT, 128))
                m3 = m3b[r][:].rearrange("p (b v) -> p b v", v=128)
                nc.vector.tensor_tensor(out=m3, in0=eq3, in1=wrev_b, op=AL.mult)
                nc.vector.tensor_reduce(out=red2[:, c0:c0 + NT], in_=m3,
                                        axis=mybir.AxisListType.X, op=AL.max)
                # pred = 127 - red  (first-index argmax, exact on ties)
                nc.vector.tensor_scalar(out=pred2[:, c0:c0 + NT],
                                        in0=red2[:, c0:c0 + NT],
                                        scalar1=-1.0, scalar2=127.0,
                                        op0=AL.mult, op1=AL.add)
                nc.vector.tensor_scalar(out=negmax2[:, c0:c0 + NT],
                                        in0=rowmax2[:, c0:c0 + NT],
                                        scalar1=-1.0, scalar2=None, op0=AL.mult)
                for n in range(NT):
                    c = c0 + n
                    nc.scalar.activation(exp_scr[n % 2][:], lg3[r][:, n, :],
                                         AF.Exp, bias=negmax2[:, c:c + 1],
                                         scale=1.0,
                                         accum_out=sumexp2[:, c:c + 1])

            nc.vector.reciprocal(pp2[:], sumexp2[:])

            # ---------------- trigram codes + pen (t = 16p + b) ----------
            # neighbours from the next partition for the b=15 boundary
            nc.vector.memset(pred_nxt[:], 0.0)
            nc.vector.memset(pp_nxt[:], 0.0)
            for r in range(R):
                nc.sync.dma_start(pred_nxt[0:127, 2 * r:2 * r + 2],
                                  pred2[1:128, NT * r:NT * r + 2])
                nc.sync.dma_start(pp_nxt[0:127, 2 * r:2 * r + 2],
                                  pp2[1:128, NT * r:NT * r + 2])

            # shift1[t] = pred[t+1], shift2[t] = pred[t+2]
            nc.vector.tensor_copy(sh1[:, 0:C2 - 1], pred2[:, 1:C2])
            nc.vector.tensor_copy(sh2[:, 0:C2 - 2], pred2[:, 2:C2])
            nc.vector.tensor_copy(ph1[:, 0:C2 - 1], pp2[:, 1:C2])
            nc.vector.tensor_copy(ph2[:, 0:C2 - 2], pp2[:, 2:C2])
            for r in range(R):
                cend = NT * (r + 1)
                nc.vector.tensor_copy(sh1[:, cend - 1:cend], pred_nxt[:, 2 * r:2 * r + 1])
                nc.vector.tensor_copy(sh2[:, cend - 2:cend - 1], pred_nxt[:, 2 * r:2 * r + 1])
                nc.vector.tensor_copy(sh2[:, cend - 1:cend], pred_nxt[:, 2 * r + 1:2 * r + 2])
                nc.vector.tensor_copy(ph1[:, cend - 1:cend], pp_nxt[:, 2 * r:2 * r + 1])
                nc.vector.tensor_copy(ph2[:, cend - 2:cend - 1], pp_nxt[:, 2 * r:2 * r + 1])
                nc.vector.tensor_copy(ph2[:, cend - 1:cend], pp_nxt[:, 2 * r + 1:2 * r + 2])

            # code = pred*16384 + sh1*128 + sh2 ; pen = pp + ph1 + ph2
            nc.vector.tensor_scalar(out=tmp_a[:], in0=pred2[:], scalar1=16384.0,
                                    scalar2=None, op0=AL.mult)
            nc.vector.scalar_tensor_tensor(out=tmp_b[:], in0=sh1[:], scalar=128.0,
                                           in1=tmp_a[:], op0=AL.mult, op1=AL.add)
            nc.vector.tensor_tensor(out=code2[:], in0=tmp_b[:], in1=sh2[:], op=AL.add)
            nc.vector.tensor_tensor(out=pen2[:], in0=pp2[:], in1=ph1[:], op=AL.add)
            nc.vector.tensor_tensor(out=pen2[:], in0=pen2[:], in1=ph2[:], op=AL.add)
            nc.vector.tensor_copy(pred_bf[:], pred2[:])

            # positions T-2, T-1 have no trigram -> bcast-side sentinel.
            # (partition 126 can't start an engine op; patch via the flat DMA
            # below: write the two cells into code_flat directly instead.)
            sentBC = scrp.tile([1, 4], F32)
            nc.vector.memset(sentBC[:], SENT_BC)

            # flatten to DRAM in t-order: flat[r*T + 16p + b] = x[p, 16r+b]
            for r in range(R):
                nc.sync.dma_start(
                    code_flat[:, r * T:(r + 1) * T].rearrange("o (a b) -> (o a) b", a=128),
                    code2[:, NT * r:NT * (r + 1)])
                nc.sync.dma_start(
                    pred_flat[:, r * T:(r + 1) * T].rearrange("o (a b) -> (o a) b", a=128),
                    pred_bf[:, NT * r:NT * (r + 1)])
                # overwrite flat code at t = T-2, T-1 with the sentinel
                nc.sync.dma_start(code_flat[:, (r + 1) * T - 2:(r + 1) * T],
                                  sentBC[:, 0:2])
            nc.sync.dma_start(pred_flat[:, R * T:R * T + 8], zeros_bf[:])

            # broadcast codes to all partitions; sentinel side columns
            for r in range(R):
                nc.gpsimd.memset(code_bcast[r][:, 0:PAD], SENT_BC)
                nc.gpsimd.memset(code_bcast[r][:, PAD + T:CB_W], SENT_BC)
                nc.sync.dma_start(
                    code_bcast[r][:, PAD:PAD + T],
                    code_flat[:, r * T:(r + 1) * T].partition_broadcast(128))

            # code_ipart[p, 16r+t] = code[r, 128t + p] via 3 bf16 transposes
            for (off, dst) in ((0, p0t), (1, p1t), (2, p2t)):
                nc.sync.dma_start_transpose(
                    dst[:], pred_flat[:, off:off + R * T]
                    .rearrange("o (q p) -> (o q) p", p=128))
            nc.vector.tensor_scalar(out=tmp_a[:], in0=p0t[:], scalar1=16384.0,
                                    scalar2=None, op0=AL.mult)
            nc.vector.scalar_tensor_tensor(out=tmp_b[:], in0=p1t[:], scalar=128.0,
                                           in1=tmp_a[:], op0=AL.mult, op1=AL.add)
            nc.vector.tensor_tensor(out=code_ipart[:], in0=tmp_b[:], in1=p2t[:],
                                    op=AL.add)
            for r in range(R):
                nc.sync.dma_start(
                    code_ipart[126:128, NT * (r + 1) - 1:NT * (r + 1)], sentI[:])

            # ---------------- pairwise match counting ----------------
            for r in range(R):
                cb = code_bcast[r]
                # diagonal blocks: eq then staircase mask, both bf16
                for t in range(NT):
                    nc.vector.tensor_scalar(
                        out=eqbig[r][:, 128 * t:128 * (t + 1)],
                        in0=cb[:, PAD + 128 * t + 3:PAD + 128 * t + 131],
                        scalar1=code_ipart[:, NT * r + t:NT * r + t + 1],
                        scalar2=None, op0=AL.is_equal)
                nc.vector.tensor_tensor(out=eqbig[r][:], in0=eqbig[r][:],
                                        in1=diagmask[:], op=AL.mult)
                # PSUM accumulation order: per tile t emit diag then main.
                # t=0's writes carry start=True (first writer of every col);
                # all later tiles' ranges are already covered by main t-1.
                for t in range(NT):
                    jlo, jhi = 128 * t + 3, min(128 * t + 131, L)
                    for (a, b2) in _bank_chunks(jlo, jhi):
                        nc.tensor.matmul(
                            counts_ps[32 * r:32 * r + 1, a:b2], ones_bf[:],
                            eqbig[r][:, 128 * t + (a - jlo):128 * t + (b2 - jlo)],
                            start=(t == 0), stop=True, skip_group_check=True)
                    # main window: j in [128t + 130, L)
                    W = L - (128 * t + 130)
                    if W <= 0:
                        continue
                    eqt = eqp.tile([128, 1920], BF16, tag="eqt", name=f"eqt{r}_{t}")
                    nc.vector.tensor_scalar(
                        out=eqt[:, 0:W],
                        in0=cb[:, PAD + 128 * t + 130:PAD + L],
                        scalar1=code_ipart[:, NT * r + t:NT * r + t + 1],
                        scalar2=None, op0=AL.is_equal)
                    jlo = 128 * t + 130
                    for (a, b2) in _bank_chunks(jlo, L):
                        nc.tensor.matmul(
                            counts_ps[32 * r:32 * r + 1, a:b2], ones_bf[:],
                            eqt[:, a - jlo:b2 - jlo],
                            start=(t == 0), stop=True, skip_group_check=True)

            # ---------------- epilogue ----------------
            for r in range(R):
                nc.scalar.copy(counts_sb[0:1, 3:L], counts_ps[32 * r:32 * r + 1, 3:L])
                nc.vector.memset(counts_sb[0:1, 0:3], 0.0)
                nc.vector.memset(counts_sb[0:1, L:T], 0.0)
                # reshape to the /16 layout: counts_div[p, b] = counts[16p + b]
                nc.sync.dma_start(counts_div[:], counts_sb[:])
                nc.vector.scalar_tensor_tensor(
                    out=junk16[:], in0=counts_div[:], scalar=1.0,
                    in1=pen2[:, NT * r:NT * (r + 1)],
                    op0=AL.mult, op1=AL.mult,
                    accum_out=s1c[:, r:r + 1])

            nc.tensor.matmul(ps_fin[:], ones_f32[:], s1c[:], start=True, stop=True)
            # final = SCALE * (ps_fin[0] + ps_fin[1])
            nc.vector.tensor_scalar(out=junk2[:], in0=ps_fin[:],
                                    scalar1=SCALE, scalar2=None,
                                    op0=AL.mult, op1=AL.add,
                                    accum_out=final_sb[:])
            nc.sync.dma_start(y_ext.ap()[:, :], final_sb[:])

    nc.compile()
    return nc


_NC_CACHE = None


def _get_nc():
    global _NC_CACHE
    if _NC_CACHE is None:
        _NC_CACHE = build_nc()
    return _NC_CACHE


def kernel(**inputs) -> np.ndarray:
    logits = np.ascontiguousarray(np.asarray(inputs["logits"], dtype=np.float32))
    assert logits.shape == (B, T, V), logits.shape
    nc = _get_nc()
    in_maps = [
        {"logits": logits[i * R:(i + 1) * R].reshape(R * T, V)}
        for i in range(N_CORES)
    ]
    res = run_bass_kernel_spmd(nc, in_maps, core_ids=list(range(N_CORES)))
    total = np.float32(0.0)
    for i in range(N_CORES):
        total = total + res.results[i]["out"][0, 0]
    return np.asarray(total, dtype=np.float32)
